# revision 4
# baseline (speedup 1.0000x reference)
"""Trainium2 Bass kernel for nn_NeuralNetwork_17360257811096 (dense_mlp).

Per-board-row MLP 5->120->84->5 with relu, L2-normalize over the last axis,
then zero out positions where the input was nonzero.

Strategy (per core, pure data parallel over 8 cores):
- Batch-major x is cast+padded to fp16 hi/lo planes (22-bit precision),
  DMA'd to DRAM, and xbar-transposed to a feature-major layout
  [128 = 4 groups x 32 board-elems, N boards] so the tiny contractions
  run on the PE with boards streaming along the free dim.
- Each linear layer runs as 3 fp16 matmuls (hi*hi, lo-weights*hi, hi*lo)
  accumulated in fp32 PSUM -> near-fp32 precision at bf16-class speed.
  Biases ride ones-rows through the matmuls.
- relu+hi-plane eviction on ScalarE, lo-plane via scalar_tensor_tensor
  ((psum max 0) - hi) on VectorE.
- Normalize: sum-of-squares via a ones-matmul, rsqrt as exp(-0.5*ln(ss))
  on ScalarE (Rsqrt table is banned), broadcast back via a matmul.
- Mask from the hi input plane (exact-zero preserving), output split to
  fp16 hi/lo planes, xbar-transposed back to batch-major, recombined to
  fp32 on-chip and DMA'd out.
"""
import numpy as np
import ml_dtypes
from contextlib import ExitStack

import concourse.bacc as bacc
import concourse.bass as bass
import concourse.mybir as mybir
import concourse.tile as tile
from concourse import bass_utils

F32 = mybir.dt.float32
F16 = mybir.dt.float16
ALU = mybir.AluOpType
ACTF = mybir.ActivationFunctionType

N_CORES = 8
B_TOTAL = 262144
BPC = B_TOTAL // N_CORES      # boards per core
CHUNK = 2048                  # boards per chunk
NCH = BPC // CHUNK            # 16 chunks
R = CHUNK // 4                # 512 free-dim columns per chunk

bf16 = ml_dtypes.bfloat16


def _hi_lo(a):
    hi = a.astype(np.float16).astype(np.float32)
    lo = (a - hi).astype(np.float16).astype(np.float32)
    return hi, lo


def pack_consts(W1, b1, W2, b2, W3, b3):
    """Host-side weight packing (layout only; all tiny)."""
    W1 = W1.astype(np.float32); b1 = b1.astype(np.float32)
    W2 = W2.astype(np.float32); b2 = b2.astype(np.float32)
    W3 = W3.astype(np.float32); b3 = b3.astype(np.float32)
    W1h, W1l = _hi_lo(W1); b1h, b1l = _hi_lo(b1)
    W2h, W2l = _hi_lo(W2); b2h, b2l = _hi_lo(b2)
    W3h, W3l = _hi_lo(W3); b3h, b3l = _hi_lo(b3)

    # L1 lhsT variants, replicated at 4 partition bases for row tiling.
    # K rows (within a 32-group): k = s'*5+i for k<25, k=25 is the ones row.
    # A: W1h + b1h ones-row; B: W1l + b1l ones-row; C: W1h, no ones row.
    w1a = np.zeros((128, 5 * 121), np.float32)
    w1b = np.zeros((128, 5 * 121), np.float32)
    w1c = np.zeros((128, 5 * 121), np.float32)
    for g in range(4):
        for s in range(5):
            for i in range(5):
                w1a[32 * g + 5 * s + i, 121 * s:121 * s + 120] = W1h[:, i]
                w1b[32 * g + 5 * s + i, 121 * s:121 * s + 120] = W1l[:, i]
                w1c[32 * g + 5 * s + i, 121 * s:121 * s + 120] = W1h[:, i]
            w1a[32 * g + 25, 121 * s:121 * s + 120] = b1h
            w1b[32 * g + 25, 121 * s:121 * s + 120] = b1l
    # col 120 of each s-block stays 0 -> psum row 120 = 0, ACT bias makes it 1.

    # L2 lhsT: [121, 85]; rows 0..119 = W2^T (y1 dim), row 120 = bias via ones.
    w2a = np.zeros((121, 85), np.float32)
    w2b = np.zeros((121, 85), np.float32)
    w2c = np.zeros((121, 85), np.float32)
    w2a[:120, :84] = W2h.T; w2a[120, :84] = b2h
    w2b[:120, :84] = W2l.T; w2b[120, :84] = b2l
    w2c[:120, :84] = W2h.T
    # col 84 all-zero -> psum2 row 84 = 0; ACT bias2 sets y2h[84]=1.

    # L3 lhsT variants per s: [85, 32]; col j=5s+o -> W3[o,:], row 84 -> b3.
    w3a = np.zeros((85, 5 * 32), np.float32)
    w3b = np.zeros((85, 5 * 32), np.float32)
    w3c = np.zeros((85, 5 * 32), np.float32)
    for s in range(5):
        for o in range(5):
            w3a[:84, 32 * s + 5 * s + o] = W3h[o, :]
            w3b[:84, 32 * s + 5 * s + o] = W3l[o, :]
            w3c[:84, 32 * s + 5 * s + o] = W3h[o, :]
            w3a[84, 32 * s + 5 * s + o] = b3h[o]
            w3b[84, 32 * s + 5 * s + o] = b3l[o]
    # NOTE col index within a 32-block must be 5s+o (absolute psum row is
    # 32g + 5s + o); the 32-col block for s uses cols [5s, 5s+5).

    # sum-of-squares lhsT [128, 20]: col m=5g+s sums rows 32g+5s+0..4
    ssw = np.zeros((128, 20), np.float32)
    for g in range(4):
        for s in range(5):
            ssw[32 * g + 5 * s:32 * g + 5 * s + 5, 5 * g + s] = 1.0
    # broadcast lhsT [20, 128] carrying the -0.5 of exp(-0.5 ln ss)
    bcw = np.zeros((20, 128), np.float32)
    for g in range(4):
        for s in range(5):
            bcw[5 * g + s, 32 * g + 5 * s:32 * g + 5 * s + 5] = -0.5

    bias1 = np.zeros((121, 1), np.float32); bias1[120, 0] = 1.0
    bias2 = np.zeros((85, 1), np.float32); bias2[84, 0] = 1.0

    f16 = np.float16
    return {
        "w1a": w1a.astype(f16), "w1b": w1b.astype(f16), "w1c": w1c.astype(f16),
        "w2a": w2a.astype(f16), "w2b": w2b.astype(f16), "w2c": w2c.astype(f16),
        "w3a": w3a.astype(f16), "w3b": w3b.astype(f16), "w3c": w3c.astype(f16),
        "ssw": ssw, "bcw": bcw, "bias1": bias1, "bias2": bias2,
        "biasln": np.full((20, 1), 1e-24, np.float32),
    }


def build_kernel():
    nc = bacc.Bacc("TRN2", target_bir_lowering=False, debug=False, num_devices=1)
    x = nc.dram_tensor("x", [BPC * 25], F32, kind="ExternalInput").ap()
    y = nc.dram_tensor("y", [BPC * 25], F32, kind="ExternalOutput").ap()
    xpadh = nc.dram_tensor("xpadh", [BPC * 32], F16, kind="Internal").ap()
    xpadl = nc.dram_tensor("xpadl", [BPC * 32], F16, kind="Internal").ap()
    w1a = nc.dram_tensor("w1a", [128, 605], F16, kind="ExternalInput").ap()
    w1b = nc.dram_tensor("w1b", [128, 605], F16, kind="ExternalInput").ap()
    w1c = nc.dram_tensor("w1c", [128, 605], F16, kind="ExternalInput").ap()
    w2a = nc.dram_tensor("w2a", [121, 85], F16, kind="ExternalInput").ap()
    w2b = nc.dram_tensor("w2b", [121, 85], F16, kind="ExternalInput").ap()
    w2c = nc.dram_tensor("w2c", [121, 85], F16, kind="ExternalInput").ap()
    w3a = nc.dram_tensor("w3a", [85, 160], F16, kind="ExternalInput").ap()
    w3b = nc.dram_tensor("w3b", [85, 160], F16, kind="ExternalInput").ap()
    w3c = nc.dram_tensor("w3c", [85, 160], F16, kind="ExternalInput").ap()
    ssw = nc.dram_tensor("ssw", [128, 20], F32, kind="ExternalInput").ap()
    bcw = nc.dram_tensor("bcw", [20, 128], F32, kind="ExternalInput").ap()
    bias1 = nc.dram_tensor("bias1", [121, 1], F32, kind="ExternalInput").ap()
    bias2 = nc.dram_tensor("bias2", [85, 1], F32, kind="ExternalInput").ap()
    biasln = nc.dram_tensor("biasln", [20, 1], F32, kind="ExternalInput").ap()

    with tile.TileContext(nc) as tc, ExitStack() as ctx:
        wpool = ctx.enter_context(tc.tile_pool(name="weights", bufs=1))
        W1A = wpool.tile([128, 605], F16, name="W1A")
        W1B = wpool.tile([128, 605], F16, name="W1B")
        W1C = wpool.tile([128, 605], F16, name="W1C")
        W2A = wpool.tile([121, 85], F16, name="W2A")
        W2B = wpool.tile([121, 85], F16, name="W2B")
        W2C = wpool.tile([121, 85], F16, name="W2C")
        W3A = wpool.tile([85, 160], F16, name="W3A")
        W3B = wpool.tile([85, 160], F16, name="W3B")
        W3C = wpool.tile([85, 160], F16, name="W3C")
        SSW = wpool.tile([128, 20], F32, name="SSW")
        BCW = wpool.tile([20, 128], F32, name="BCW")
        BIAS1 = wpool.tile([121, 1], F32, name="BIAS1")
        BIAS2 = wpool.tile([85, 1], F32, name="BIAS2")
        BIASLN = wpool.tile([20, 1], F32, name="BIASLN")
        for t, d in [(W1A, w1a), (W1B, w1b), (W1C, w1c), (W2A, w2a),
                     (W2B, w2b), (W2C, w2c), (W3A, w3a), (W3B, w3b),
                     (W3C, w3c), (SSW, ssw), (BCW, bcw), (BIAS1, bias1),
                     (BIAS2, bias2), (BIASLN, biasln)]:
            nc.sync.dma_start(t[:], d)

        apool = ctx.enter_context(tc.tile_pool(name="stagea", bufs=2))
        tpool = ctx.enter_context(tc.tile_pool(name="xt", bufs=2))
        hpool = ctx.enter_context(tc.tile_pool(name="hidden", bufs=2))
        npool = ctx.enter_context(tc.tile_pool(name="norm", bufs=2))
        opool = ctx.enter_context(tc.tile_pool(name="outp", bufs=2))
        p1pool = ctx.enter_context(tc.tile_pool(name="ps1", bufs=1, space="PSUM"))
        p2pool = ctx.enter_context(tc.tile_pool(name="ps2", bufs=1, space="PSUM"))
        p3pool = ctx.enter_context(tc.tile_pool(name="ps3", bufs=1, space="PSUM"))

        for c in range(NCH):
            cb = c * CHUNK  # chunk board base
            # ---- stage A: cast to fp16 hi/lo planes, pad 25->32 ----
            xnat = apool.tile([128, 512], F32, name="xnat", tag="xnat")
            xv = xnat[:].rearrange("p (k e) -> p k e", e=32)
            nc.vector.memset(xv[:, :, 25:32], 0.0)
            # partition p holds boards cb+16p .. cb+16p+15
            xin = x[cb * 25:(cb + CHUNK) * 25].rearrange(
                "(p k e) -> p k e", p=128, k=16)
            nc.gpsimd.dma_start(xv[:, :, 0:25], xin)
            xh = apool.tile([128, 512], F16, name="xh", tag="xh")
            nc.vector.tensor_copy(xh[:], xnat[:])
            xhv = xh[:].rearrange("p (k e) -> p k e", e=32)
            nc.vector.memset(xhv[:, :, 25:26], 1.0)  # ones row for L1 bias
            xl = apool.tile([128, 512], F16, name="xl", tag="xl")
            nc.vector.scalar_tensor_tensor(
                xl[:], xnat[:], 0.0, xh[:], op0=ALU.bypass, op1=ALU.subtract)
            xpadh_c = xpadh[cb * 32:(cb + CHUNK) * 32].rearrange(
                "(p f) -> p f", p=128)
            xpadl_c = xpadl[cb * 32:(cb + CHUNK) * 32].rearrange(
                "(p f) -> p f", p=128)
            nc.sync.dma_start(xpadh_c, xh[:])
            nc.sync.dma_start(xpadl_c, xl[:])
            # ---- transpose to feature-major [128, 512] ----
            XTh = tpool.tile([128, 512], F16, name="XTh", tag="XTh")
            XTl = tpool.tile([128, 512], F16, name="XTl", tag="XTl")
            trh = xpadh[cb * 32:(cb + CHUNK) * 32].rearrange(
                "(r q) -> r q", q=128)
            trl = xpadl[cb * 32:(cb + CHUNK) * 32].rearrange(
                "(r q) -> r q", q=128)
            nc.sync.dma_start_transpose(XTh[:], trh)
            nc.sync.dma_start_transpose(XTl[:], trl)
            # XT[32g + (5s+i), r] = x[board 4r+g, s, i]; row 32g+25 = 1.0 (hi)

            p3 = p3pool.tile([128, 2048], F32, name="p3", tag="p3")
            y3f = npool.tile([128, 512], F32, name="y3f", tag="y3f")

            for s in range(5):
                # ---- L1: 3 fp16 plane-matmuls, 2-group row tiling ----
                y1h = hpool.tile([121, 2048], F16, name="y1h", tag="y1h")
                y1l = hpool.tile([120, 2048], F16, name="y1l", tag="y1l")
                for half in range(2):
                    p1 = p1pool.tile([121, 1024], F32, name="p1", tag="p1")
                    for pl, (W, XT, K) in enumerate(
                            [(W1A, XTh, 26), (W1B, XTh, 26), (W1C, XTl, 25)]):
                        for j in range(2):
                            g = 2 * half + j
                            nc.tensor.matmul(
                                p1[:, 512 * j:512 * (j + 1)],
                                W[32 * g:32 * g + K, 121 * s:121 * (s + 1)],
                                XT[32 * g:32 * g + K, :],
                                start=(pl == 0), stop=(pl == 2),
                                tile_position=(32 * g, 0))
                    hs = slice(1024 * half, 1024 * (half + 1))
                    nc.scalar.activation(
                        y1h[:, hs], p1[:], ACTF.Relu, bias=BIAS1[:])
                    nc.vector.scalar_tensor_tensor(
                        y1l[:, hs], p1[0:120, :], 0.0, y1h[0:120, hs],
                        op0=ALU.max, op1=ALU.subtract)
                # ---- L2 ----
                y2h = hpool.tile([85, 2048], F16, name="y2h", tag="y2h")
                y2l = hpool.tile([84, 2048], F16, name="y2l", tag="y2l")
                for half in range(2):
                    p2 = p2pool.tile([85, 1024], F32, name="p2", tag="p2")
                    for j in range(2):
                        g = 2 * half + j
                        gs = slice(512 * g, 512 * (g + 1))
                        js = slice(512 * j, 512 * (j + 1))
                        nc.tensor.matmul(p2[:, js], W2A[:], y1h[:, gs],
                                         start=True, stop=False)
                        nc.tensor.matmul(p2[:, js], W2B[:], y1h[:, gs],
                                         start=False, stop=False)
                        nc.tensor.matmul(p2[:, js], W2C[0:120, :], y1l[:, gs],
                                         start=False, stop=True)
                    hs = slice(1024 * half, 1024 * (half + 1))
                    nc.scalar.activation(
                        y2h[:, hs], p2[:], ACTF.Relu, bias=BIAS2[:])
                    nc.vector.scalar_tensor_tensor(
                        y2l[:, hs], p2[0:84, :], 0.0, y2h[0:84, hs],
                        op0=ALU.max, op1=ALU.subtract)
                # ---- L3: accumulate into p3, 4-group col tiling ----
                for pl, (W, yt, K) in enumerate(
                        [(W3A, y2h, 85), (W3B, y2h, 85), (W3C, y2l, 84)]):
                    for g in range(4):
                        nc.tensor.matmul(
                            p3[32 * g:32 * g + 32, 512 * g:512 * (g + 1)],
                            W[0:K, 32 * s:32 * (s + 1)],
                            yt[0:K, 512 * g:512 * (g + 1)],
                            start=(s == 0 and pl == 0),
                            stop=(s == 4 and pl == 2),
                            tile_position=(0, 32 * g))

            # ---- normalize + mask ----
            for g in range(4):
                nc.vector.tensor_scalar_max(
                    y3f[32 * g:32 * g + 32, :],
                    p3[32 * g:32 * g + 32, 512 * g:512 * (g + 1)], 0.0)
            y3sq = npool.tile([128, 512], F32, name="y3sq", tag="y3sq")
            nc.scalar.square(y3sq[:], y3f[:])
            pss = p3pool.tile([20, 512], F32, name="pss", tag="p3")
            nc.tensor.matmul(pss[:], SSW[:], y3sq[:], start=True, stop=True)
            lnss = npool.tile([20, 512], F32, name="lnss", tag="lnss")
            nc.scalar.activation(lnss[:], pss[:], ACTF.Ln, bias=BIASLN[:])
            pbc = p3pool.tile([128, 512], F32, name="pbc", tag="p3")
            for j in range(2):
                nc.tensor.matmul(pbc[64 * j:64 * (j + 1), :],
                                 BCW[:, 64 * j:64 * (j + 1)], lnss[:],
                                 start=True, stop=True)
            rsb = npool.tile([128, 512], F32, name="rsb", tag="rsb")
            nc.scalar.activation(rsb[:], pbc[:], ACTF.Exp)
            m01 = npool.tile([128, 512], F16, name="m01", tag="m01")
            nc.vector.tensor_scalar(m01[:], XTh[:], 0.0, None, op0=ALU.is_equal)
            rsm = npool.tile([128, 512], F32, name="rsm", tag="rsm")
            nc.vector.tensor_tensor(rsm[:], rsb[:], m01[:], op=ALU.mult)
            yn32 = npool.tile([128, 512], F32, name="yn32", tag="yn32")
            nc.vector.tensor_tensor(yn32[:], y3f[:], rsm[:], op=ALU.mult)
            yh = opool.tile([128, 512], F16, name="yh", tag="yh")
            nc.vector.tensor_copy(yh[:], yn32[:])
            yl = opool.tile([128, 512], F16, name="yl", tag="yl")
            nc.vector.scalar_tensor_tensor(
                yl[:], yn32[:], 0.0, yh[:], op0=ALU.bypass, op1=ALU.subtract)
            # ---- transpose back to batch-major, recombine, store ----
            Yh = opool.tile([128, 512], F16, name="Yh", tag="Yh")
            Yl = opool.tile([128, 512], F16, name="Yl", tag="Yl")
            for j in range(4):
                js = slice(128 * j, 128 * (j + 1))
                nc.sync.dma_start_transpose(Yh[:, js], yh[:, js])
                nc.sync.dma_start_transpose(Yl[:, js], yl[:, js])
            yn = opool.tile([128, 512], F32, name="yn", tag="yn")
            nc.vector.tensor_tensor(yn[:], Yh[:], Yl[:], op=ALU.add)
            # Yn[a, 128j + 32g + e] = board(cb + 512j + 4a + g) elem e
            ynv = yn[:].rearrange("a (j g e) -> a j g e", j=4, g=4)
            for j in range(4):
                jb = cb + 512 * j
                yout = y[jb * 25:(jb + 512) * 25].rearrange(
                    "(a g e) -> a g e", a=128, g=4)
                nc.gpsimd.dma_start(yout, ynv[:, j, :, 0:25])

    nc.compile()
    return nc


_cached = None


def _get_nc():
    global _cached
    if _cached is None:
        _cached = build_kernel()
    return _cached


def kernel(x, W1, b1, W2, b2, W3, b3):
    x = np.ascontiguousarray(np.asarray(x, dtype=np.float32))
    consts = pack_consts(np.asarray(W1), np.asarray(b1), np.asarray(W2),
                         np.asarray(b2), np.asarray(W3), np.asarray(b3))
    nc = _get_nc()
    in_maps = []
    for c in range(N_CORES):
        m = {"x": x[c * BPC:(c + 1) * BPC].reshape(-1)}
        m.update(consts)
        in_maps.append(m)
    res = bass_utils.run_bass_kernel_spmd(nc, in_maps, core_ids=list(range(N_CORES)))
    out = np.concatenate(
        [res.results[c]["y"].reshape(BPC, 5, 5) for c in range(N_CORES)], axis=0)
    return out.astype(np.float32)


if __name__ == "__main__":
    nc = build_kernel()
    print("kernel built OK")
